# revision 16
# baseline (speedup 1.0000x reference)
"""MultiHeadTEAttention TRN2 kernel — 8-core SPMD, one batch element per core.

Architecture (per core, batch m):
  - Transposed-K ("flash") layout: dots^T[k,q] per head; softmax over the
    partition dim via PE ones-matmul column sums (values bounded, no max pass).
  - Kernel-MLP bias: R[k,(c,q)] = relu(b[k,c] + a[q,c]) via DVE tensor_scalar
    (fused add+relu, bf16); DMA partition-shuffle to [(k_lo,c),(q)] layout;
    PE contraction with a fixed E-matrix E[(k_lo,c),(h,k_lo)] = kw2[c,h];
    exp on ACT straight out of PSUM; DMA partition-shuffle back to per-head
    [k,q]; multiplied into exp(token_dots) on DVE.
  - AV and token-dots contract on PE in bf16; fp32 PSUM accumulation.
"""

import numpy as np

import concourse.bass as bass
import concourse.mybir as mybir
import concourse.tile as tile
from concourse import bacc, bass_utils

F32 = mybir.dt.float32
BF16 = mybir.dt.bfloat16
AX = mybir.AluOpType

M, NQ, NKV, DX, DT = 8, 1024, 1024, 512, 2
H, HD = 8, 64
INNER = H * HD          # 512
KHID = 16               # c
SCALE = HD ** -0.5
P = 128
NKT = NKV // P          # 8 k-tiles
NQC = 2                 # q chunks of 512
QC = NQ // NQC          # 512
NIT = INNER // P        # 4 inner tiles
NDXT = DX // P          # 4 dx tiles


def build_kernel(nc: bass.Bass):
    # ---- DRAM I/O ----
    d_xq = nc.dram_tensor("xq", [NQ, DX], F32, kind="ExternalInput").ap()
    d_xk = nc.dram_tensor("xk", [NKV, DX], F32, kind="ExternalInput").ap()
    d_xv = nc.dram_tensor("xv", [NKV, DX], F32, kind="ExternalInput").ap()
    d_tq = nc.dram_tensor("tq", [NQ, DT], F32, kind="ExternalInput").ap()
    d_tk = nc.dram_tensor("tk", [NKV, DT], F32, kind="ExternalInput").ap()
    d_wq = nc.dram_tensor("w_q", [DX, INNER], F32, kind="ExternalInput").ap()
    d_wk = nc.dram_tensor("w_k", [DX, INNER], F32, kind="ExternalInput").ap()
    d_wv = nc.dram_tensor("w_v", [DX, INNER], F32, kind="ExternalInput").ap()
    d_wout = nc.dram_tensor("w_out", [INNER, DX], F32, kind="ExternalInput").ap()
    d_bout = nc.dram_tensor("b_out", [DX], F32, kind="ExternalInput").ap()
    d_kw1 = nc.dram_tensor("kw1", [DT, KHID], F32, kind="ExternalInput").ap()
    d_kb1 = nc.dram_tensor("kb1", [KHID], F32, kind="ExternalInput").ap()
    d_kw2 = nc.dram_tensor("kw2", [KHID, H], F32, kind="ExternalInput").ap()
    d_kb2 = nc.dram_tensor("kb2", [H], F32, kind="ExternalInput").ap()
    d_out = nc.dram_tensor("out", [NQ, DX], F32, kind="ExternalOutput").ap()

    with tile.TileContext(nc) as tc:
        _body(tc, d_xq, d_xk, d_xv, d_tq, d_tk, d_wq, d_wk, d_wv, d_wout,
              d_bout, d_kw1, d_kb1, d_kw2, d_kb2, d_out)
    return nc


def _unit(ap):
    return bass.AP(tensor=ap.tensor, offset=ap.offset, ap=list(ap.ap) + [[1, 1]])


def _col(ap1d):
    return bass.AP(tensor=ap1d.tensor, offset=ap1d.offset,
                   ap=[list(ap1d.ap[0]), [1, 1]])


def _bcast(ap_row, parts):
    # ap_row: [1, N]-ish AP -> broadcast over `parts` partitions via step-0
    return bass.AP(tensor=ap_row.tensor, offset=ap_row.offset,
                   ap=[[0, parts]] + list(ap_row.ap[1:]))


def _body(tc, d_xq, d_xk, d_xv, d_tq, d_tk, d_wq, d_wk, d_wv, d_wout,
          d_bout, d_kw1, d_kb1, d_kw2, d_kb2, d_out):
    nc = tc.nc
    import contextlib
    ctx = contextlib.ExitStack()
    persist = ctx.enter_context(tc.tile_pool(name="persist", bufs=1))
    work = ctx.enter_context(tc.tile_pool(name="work", bufs=3))
    ctx0 = contextlib.ExitStack()
    stage = ctx0.enter_context(tc.tile_pool(name="stage", bufs=2))
    xw_pool = ctx0.enter_context(tc.tile_pool(name="xw", bufs=1))
    psum_w = ctx0.enter_context(tc.tile_pool(name="psum_w", bufs=2, space="PSUM"))
    psum_a = ctx0.enter_context(tc.tile_pool(name="psum_a", bufs=1, space="PSUM"))

    # ================= phase 0: constants & small precompute =================
    # tqT/tkT [2, 1024] via swapped-AP DMA (tiny)
    tqT = persist.tile([DT, NQ], F32)
    tkT = persist.tile([DT, NKV], F32)
    nc.sync.dma_start(out=tqT[:, :], in_=_unit(d_tq.rearrange("q t -> t q")))
    nc.sync.dma_start(out=tkT[:, :], in_=_unit(d_tk.rearrange("k t -> t k")))
    kw1_sb = persist.tile([DT, KHID], F32)
    nc.sync.dma_start(out=kw1_sb[:, :], in_=d_kw1[:, :])
    kb1_sb = persist.tile([KHID, 1], F32)
    nc.sync.dma_start(out=kb1_sb[:, :], in_=_col(d_kb1))
    kw2_sb = persist.tile([KHID, H], F32)
    nc.sync.dma_start(out=kw2_sb[:, :], in_=d_kw2[:, :])
    kb2_sb = persist.tile([H, 1], F32)
    nc.sync.dma_start(out=kb2_sb[:, :], in_=_col(d_kb2))
    bout_bc = persist.tile([P, DX], F32)
    nc.sync.dma_start(
        out=bout_bc[:, :],
        in_=bass.AP(tensor=d_bout.tensor, offset=d_bout.offset,
                    ap=[[0, P], [1, DX]]))

    tqT_bf = persist.tile([DT, NQ], BF16)
    tkT_bf = persist.tile([DT, NKV], BF16)
    kw1_bf = persist.tile([DT, KHID], BF16)
    nc.vector.tensor_copy(tqT_bf[:, :], tqT[:, :])
    nc.vector.tensor_copy(tkT_bf[:, :], tkT[:, :])
    nc.vector.tensor_copy(kw1_bf[:, :], kw1_sb[:, :])
    kw2_bf = persist.tile([KHID, H], BF16)
    nc.vector.tensor_copy(kw2_bf[:, :], kw2_sb[:, :])

    # kb2 pattern tile: partition (half,h,klo) -> kb2[h]  (DRAM step-0 bcast)
    kb2_pat = persist.tile([P, 1], F32)
    for half in range(2):
        for h in range(H):
            nc.sync.dma_start(
                out=kb2_pat[half * 64 + h * 8: half * 64 + h * 8 + 8, :],
                in_=bass.AP(tensor=d_kb2.tensor, offset=d_kb2.offset + h,
                            ap=[[0, 8], [1, 1]]))

    # E matrix [128=(klo,c), 64=(h,klo)]: E[klo*16+c, h*8+klo] = kw2[c,h]
    E_sb = persist.tile([P, 64], BF16)
    nc.vector.memset(E_sb[:, :], 0.0)
    for klo in range(8):
        nc.sync.dma_start(
            out=_unit(E_sb[klo * 16: klo * 16 + 16, klo::8]),
            in_=_unit(kw2_bf[:, :]))

    ones_sb = persist.tile([P, 1], BF16)
    nc.vector.memset(ones_sb[:, :], 1.0)
    zeros_sb = persist.tile([P, P], BF16)
    nc.vector.memset(zeros_sb[:, :], 0.0)

    # aT[c,q] = kw1^T tqT + kb1 (bf16); b[k,c] = -(tk kw1) (f32, per k-tile)
    aT_ps = psum_a.tile([KHID, NQ], F32)
    for j in range(NQ // 512):
        nc.tensor.matmul(aT_ps[:, j * 512:(j + 1) * 512], kw1_bf[:, :],
                         tqT_bf[:, j * 512:(j + 1) * 512], start=True, stop=True)
    aT_bf = persist.tile([KHID, NQ], BF16)
    nc.scalar.activation(aT_bf[:, :], aT_ps[:, :],
                         mybir.ActivationFunctionType.Identity,
                         bias=kb1_sb[:, :], scale=1.0)
    dram_pool = ctx.enter_context(tc.tile_pool(name="drsc", bufs=1, space="DRAM"))
    aT_dram = dram_pool.tile([KHID, NQ], BF16)
    nc.sync.dma_start(out=aT_dram[:, :], in_=aT_bf[:, :])

    b_sb = persist.tile([P, NKT, KHID], F32)
    for kt in range(NKT):
        b_ps = psum_w.tile([P, KHID], F32, tag="b_ps")
        nc.tensor.matmul(b_ps[:, :],
                         tkT_bf[:, kt * P:(kt + 1) * P], kw1_bf[:, :],
                         start=True, stop=True)
        nc.scalar.activation(b_sb[:, kt, :], b_ps[:, :],
                             mybir.ActivationFunctionType.Copy, scale=-1.0)

    # ================= phase 1: projections =================
    # load weights, cast bf16
    def load_w(dram, name):
        w_f = stage.tile([P, NDXT, INNER], F32, tag="w_f")
        nc.sync.dma_start(out=w_f[:, :, :],
                          in_=dram.rearrange("(t p) i -> p t i", p=P))
        w_b = (persist if name == "wout_bf" else xw_pool).tile(
            [P, NDXT, INNER], BF16, tag=name)
        for t in range(NDXT):
            nc.vector.tensor_copy(w_b[:, t, :], w_f[:, t, :])
        return w_b

    wq_bf = load_w(d_wq, "wq_bf")
    wk_bf = load_w(d_wk, "wk_bf")
    wv_bf = load_w(d_wv, "wv_bf")
    wout_bf = load_w(d_wout, "wout_bf")

    # load x straight, cast bf16, DMA-transpose to xT_bf [128, 4, 1024]
    def load_xT(dram, name):
        xT = xw_pool.tile([P, NDXT, NQ], BF16, tag=name)
        for qt in range(NQ // P):
            x_f = stage.tile([P, DX], F32, tag="x_f")
            nc.sync.dma_start(out=x_f[:, :], in_=dram[qt * P:(qt + 1) * P, :])
            x_b = stage.tile([P, DX], BF16, tag="x_b")
            nc.vector.tensor_copy(x_b[:, :], x_f[:, :])
            for dt_ in range(NDXT):
                nc.sync.dma_start_transpose(
                    out=xT[:, dt_, qt * P:(qt + 1) * P],
                    in_=x_b[:, dt_ * P:(dt_ + 1) * P])
        return xT

    xqT_bf = load_xT(d_xq, "xqT_bf")
    xkT_bf = load_xT(d_xk, "xkT_bf")
    xvT_bf = load_xT(d_xv, "xvT_bf")

    # qT/kT [128, 4, 1024] bf16 (qT folded with SCALE); v [128, 8, 512] bf16
    qT_bf = persist.tile([P, NIT, NQ], BF16)
    kT_bf = persist.tile([P, NIT, NKV], BF16)
    v_bf = [persist.tile([P, INNER], BF16, tag=f"v_bf{kt}", name=f"v_bf{kt}")
            for kt in range(NKT)]
    for it in range(NIT):
        for j in range(NQ // 512):
            pq = psum_w.tile([P, 512], F32, tag="proj_ps")
            pk = psum_w.tile([P, 512], F32, tag="proj_ps")
            for dt_ in range(NDXT):
                nc.tensor.matmul(pq[:, :], wq_bf[:, dt_, it * P:(it + 1) * P],
                                 xqT_bf[:, dt_, j * 512:(j + 1) * 512],
                                 start=(dt_ == 0), stop=(dt_ == NDXT - 1))
            for dt_ in range(NDXT):
                nc.tensor.matmul(pk[:, :], wk_bf[:, dt_, it * P:(it + 1) * P],
                                 xkT_bf[:, dt_, j * 512:(j + 1) * 512],
                                 start=(dt_ == 0), stop=(dt_ == NDXT - 1))
            nc.scalar.activation(qT_bf[:, it, j * 512:(j + 1) * 512], pq[:, :],
                                 mybir.ActivationFunctionType.Copy, scale=SCALE)
            nc.scalar.activation(kT_bf[:, it, j * 512:(j + 1) * 512], pk[:, :],
                                 mybir.ActivationFunctionType.Copy, scale=1.0)
    for kt in range(NKT):
        pv = psum_w.tile([P, INNER], F32, tag="proj_ps")
        for dt_ in range(NDXT):
            nc.tensor.matmul(pv[:, :], xvT_bf[:, dt_, kt * P:(kt + 1) * P],
                             wv_bf[:, dt_, :],
                             start=(dt_ == 0), stop=(dt_ == NDXT - 1))
        nc.vector.tensor_copy(v_bf[kt][:, :], pv[:, :])

    # ================= phase 2: attention per q-chunk =================
    ctx0.close()   # free phase-0/1 transient SBUF + PSUM pools
    ctx2 = contextlib.ExitStack()
    psum_acc = ctx2.enter_context(tc.tile_pool(name="psum_acc", bufs=1, space="PSUM"))
    rshuf_pool = ctx2.enter_context(tc.tile_pool(name="rshuf", bufs=2))
    rk_pool = ctx2.enter_context(tc.tile_pool(name="rk", bufs=2))
    eb_pool = ctx2.enter_context(tc.tile_pool(name="eb", bufs=2))
    ebs_pool = ctx2.enter_context(tc.tile_pool(name="ebs", bufs=2))
    et_pool = ctx2.enter_context(tc.tile_pool(name="et", bufs=4))
    p_pool = ctx2.enter_context(tc.tile_pool(name="pp", bufs=4))
    abc_pool = ctx2.enter_context(tc.tile_pool(name="abc", bufs=1))
    psum_d = ctx2.enter_context(tc.tile_pool(name="psum_d", bufs=1, space="PSUM"))

    outT_sb = persist.tile([P, NIT, NQ], BF16)   # [ (h-pair d), it, q ]

    for qc in range(NQC):
        q0 = qc * QC
        # A_bc [128, c, 512] bf16 broadcast of aT
        A_bc = abc_pool.tile([P, KHID, QC], BF16)
        for c in range(KHID):
            nc.sync.dma_start(
                out=A_bc[:, c, :],
                in_=bass.AP(tensor=aT_dram.tensor,
                            offset=aT_dram[c:c + 1, q0:q0 + QC].offset,
                            ap=[[0, P], [1, QC]]))

        # persistent accumulators for this q-chunk
        av_ps = psum_acc.tile([P, NIT, QC], F32, tag="av")      # 4 banks, 2 heads each
        z_ps = psum_acc.tile([P, 2, QC], F32, tag="z")          # 2 banks, 4 slots each
        for it in range(NIT):
            nc.tensor.matmul(av_ps[:, it, :], zeros_sb[:, :],
                             wout_bf[:, 0, :], start=True, stop=False)
        for zb in range(2):
            nc.tensor.matmul(z_ps[:, zb, :], zeros_sb[:, :],
                             wout_bf[:, 0, :], start=True, stop=False)

        for kt in range(NKT):
            # ---- R production: R_k[k, c, q] = relu(A_bc + b) ----
            R_k = rk_pool.tile([P, KHID, QC], BF16)
            for c in range(KHID):
                nc.vector.tensor_scalar(
                    out=R_k[:, c, :], in0=A_bc[:, c, :],
                    scalar1=b_sb[:, kt, c:c + 1], scalar2=0.0,
                    op0=AX.add, op1=AX.max)
            # ---- R shuffle: -> R_shuf[(klo,c), kgrp, q] ----
            R_shuf = rshuf_pool.tile([P, 16, QC], BF16)
            for kg in range(16):
                nc.sync.dma_start(
                    out=R_shuf[:, kg, :],
                    in_=R_k[kg * 8:(kg + 1) * 8, :, :])
            # ---- E-mm pairs + exp -> E_b_sb [128=(half,h,klo), pair, q] ----
            E_b = eb_pool.tile([P, 8, QC], BF16)
            for pair in range(8):
                bias_ps = psum_d.tile([P, QC], F32, tag="bias_ps")
                nc.tensor.matmul(bias_ps[0:64, :], E_sb[:, :],
                                 R_shuf[:, 2 * pair, :], start=True, stop=True)
                nc.tensor.matmul(bias_ps[64:128, :], E_sb[:, :],
                                 R_shuf[:, 2 * pair + 1, :], start=True, stop=True,
                                 tile_position=(0, 64))
                nc.scalar.activation(E_b[:, pair, :], bias_ps[:, :],
                                     mybir.ActivationFunctionType.Exp,
                                     bias=kb2_pat[:, :], scale=1.0)
            # ---- E_b shuffle -> E_b_shuf [k, h, q] ----
            E_b_shuf = ebs_pool.tile([P, H, QC], BF16)
            for h in range(H):
                for pair in range(8):
                    for half in range(2):
                        nc.sync.dma_start(
                            out=E_b_shuf[pair * 16 + half * 8:
                                         pair * 16 + half * 8 + 8, h, :],
                            in_=E_b[half * 64 + h * 8: half * 64 + h * 8 + 8,
                                    pair, :])
            # ---- per head: QK, exp, P=Et*Eb, AV, Z ----
            for h in range(H):
                it = h // 2
                r0 = (h % 2) * 64
                dots_ps = psum_d.tile([P, QC], F32, tag="dots_ps")
                nc.tensor.matmul(
                    dots_ps[:, :],
                    kT_bf[r0:r0 + 64, it, kt * P:(kt + 1) * P],
                    qT_bf[r0:r0 + 64, it, q0:q0 + QC],
                    start=True, stop=True)
                E_t = et_pool.tile([P, QC], BF16)
                nc.scalar.activation(E_t[:, :], dots_ps[:, :],
                                     mybir.ActivationFunctionType.Exp)
                P_sb = p_pool.tile([P, QC], BF16)
                nc.vector.tensor_mul(P_sb[:, :], E_t[:, :], E_b_shuf[:, h, :])
                nc.tensor.matmul(
                    av_ps[r0:r0 + 64, it, :],
                    v_bf[kt][:, h * 64:(h + 1) * 64], P_sb[:, :],
                    start=False, stop=False,
                    tile_position=(0, r0) if r0 else None)
                zslot = h % 4
                nc.tensor.matmul(
                    z_ps[zslot * 32: zslot * 32 + 1, h // 4, :],
                    ones_sb[:, :], P_sb[:, :],
                    start=False, stop=False,
                    tile_position=(0, zslot * 32) if zslot else None)

        for it in range(NIT):
            nc.tensor.matmul(av_ps[:, it, :], zeros_sb[:, :],
                             wout_bf[:, 0, :], start=False, stop=True)
        for zb in range(2):
            nc.tensor.matmul(z_ps[:, zb, :], zeros_sb[:, :],
                             wout_bf[:, 0, :], start=False, stop=True)
        # ---- normalize: outT = av / Z (recip -> DRAM bounce -> bcast) ----
        zr_dram = dram_pool.tile([H, QC], F32, tag="zr_dram")
        for h in range(H):
            zr_t = work.tile([1, QC], F32, tag="zr_t")
            nc.vector.reciprocal(zr_t[:, :],
                                 z_ps[(h % 4) * 32:(h % 4) * 32 + 1, h // 4, :])
            nc.sync.dma_start(out=zr_dram[h:h + 1, :], in_=zr_t[:, :])
        for h in range(H):
            zr_bc = work.tile([64, QC], F32, tag="zr_bc")
            nc.sync.dma_start(
                out=zr_bc[:, :],
                in_=bass.AP(tensor=zr_dram.tensor,
                            offset=zr_dram[h:h + 1, :].offset,
                            ap=[[0, 64], [1, QC]]))
            r0 = (h % 2) * 64
            nc.vector.tensor_mul(outT_sb[r0:r0 + 64, h // 2, q0:q0 + QC],
                                 av_ps[r0:r0 + 64, h // 2, :], zr_bc[:, :])

    # ================= phase 3: output projection =================
    ctx2.close()   # free phase-2 pools
    psum_o = ctx.enter_context(tc.tile_pool(name="psum_o", bufs=2, space="PSUM"))
    for qt in range(NQ // P):
        op = psum_o.tile([P, DX], F32, tag="op")
        for it in range(NIT):
            nc.tensor.matmul(op[:, :],
                             outT_sb[:, it, qt * P:(qt + 1) * P],
                             wout_bf[:, it, :],
                             start=(it == 0), stop=(it == NIT - 1))
        o_sb = work.tile([P, DX], F32, tag="o_sb")
        nc.vector.tensor_add(o_sb[:, :], op[:, :], bout_bc[:, :])
        nc.sync.dma_start(out=d_out[qt * P:(qt + 1) * P, :], in_=o_sb[:, :])

    ctx.close()


_NC_CACHE = None


def _get_nc():
    global _NC_CACHE
    if _NC_CACHE is None:
        nc = bacc.Bacc("TRN2", target_bir_lowering=False, debug=False,
                       enable_asserts=False, num_devices=M)
        build_kernel(nc)
        nc.compile()
        _NC_CACHE = nc
    return _NC_CACHE


def kernel(**inputs):
    nc = _get_nc()
    shared = {n: np.ascontiguousarray(np.asarray(inputs[n], dtype=np.float32))
              for n in ["w_q", "w_k", "w_v", "w_out", "b_out",
                        "kw1", "kb1", "kw2", "kb2"]}
    in_maps = []
    for i in range(M):
        m = dict(shared)
        for n in ["xq", "xk", "xv", "tq", "tk"]:
            m[n] = np.ascontiguousarray(np.asarray(inputs[n][i], dtype=np.float32))
        in_maps.append(m)
    res = bass_utils.run_bass_kernel_spmd(nc, in_maps, core_ids=list(range(M)))
    out = np.stack([res.results[i]["out"] for i in range(M)], axis=0)
    return out.astype(np.float32)


if __name__ == "__main__":
    import reference
    inputs = {k: np.asarray(v) for k, v in reference.setup_inputs().items()}
    out = kernel(**inputs)
    print("out", out.shape, out.dtype)


# revision 19
# speedup vs baseline: 1042.8841x; 1042.8841x over previous
"""MultiHeadTEAttention TRN2 kernel — 8-core SPMD, one batch element per core.

Architecture (per core, batch m):
  - Transposed-K ("flash") layout: dots^T[k,q] per head; softmax over the
    partition dim via PE ones-matmul column sums (values bounded, no max pass).
  - Kernel-MLP bias: R[k,(c,q)] = relu(b[k,c] + a[q,c]) via DVE tensor_scalar
    (fused add+relu, bf16); DMA partition-shuffle to [(k_lo,c),(q)] layout;
    PE contraction with a fixed E-matrix E[(k_lo,c),(h,k_lo)] = kw2[c,h];
    exp on ACT straight out of PSUM; DMA partition-shuffle back to per-head
    [k,q]; multiplied into exp(token_dots) on DVE.
  - AV and token-dots contract on PE in bf16; fp32 PSUM accumulation.
"""

import numpy as np

import concourse.bass as bass
import concourse.mybir as mybir
import concourse.tile as tile
from concourse import bacc, bass_utils

F32 = mybir.dt.float32
BF16 = mybir.dt.bfloat16
AX = mybir.AluOpType

M, NQ, NKV, DX, DT = 8, 1024, 1024, 512, 2
H, HD = 8, 64
INNER = H * HD          # 512
KHID = 16               # c
SCALE = HD ** -0.5
P = 128
NKT = NKV // P          # 8 k-tiles
NQC = 2                 # q chunks of 512
QC = NQ // NQC          # 512
NIT = INNER // P        # 4 inner tiles
NDXT = DX // P          # 4 dx tiles


def build_kernel(nc: bass.Bass):
    # ---- DRAM I/O ----
    d_xq = nc.dram_tensor("xq", [NQ, DX], F32, kind="ExternalInput").ap()
    d_xk = nc.dram_tensor("xk", [NKV, DX], F32, kind="ExternalInput").ap()
    d_xv = nc.dram_tensor("xv", [NKV, DX], F32, kind="ExternalInput").ap()
    d_tq = nc.dram_tensor("tq", [NQ, DT], F32, kind="ExternalInput").ap()
    d_tk = nc.dram_tensor("tk", [NKV, DT], F32, kind="ExternalInput").ap()
    d_wq = nc.dram_tensor("w_q", [DX, INNER], F32, kind="ExternalInput").ap()
    d_wk = nc.dram_tensor("w_k", [DX, INNER], F32, kind="ExternalInput").ap()
    d_wv = nc.dram_tensor("w_v", [DX, INNER], F32, kind="ExternalInput").ap()
    d_wout = nc.dram_tensor("w_out", [INNER, DX], F32, kind="ExternalInput").ap()
    d_bout = nc.dram_tensor("b_out", [DX], F32, kind="ExternalInput").ap()
    d_kw1 = nc.dram_tensor("kw1", [DT, KHID], F32, kind="ExternalInput").ap()
    d_kb1 = nc.dram_tensor("kb1", [KHID], F32, kind="ExternalInput").ap()
    d_kw2 = nc.dram_tensor("kw2", [KHID, H], F32, kind="ExternalInput").ap()
    d_kb2 = nc.dram_tensor("kb2", [H], F32, kind="ExternalInput").ap()
    d_out = nc.dram_tensor("out", [NQ, DX], F32, kind="ExternalOutput").ap()

    with tile.TileContext(nc) as tc:
        _body(tc, d_xq, d_xk, d_xv, d_tq, d_tk, d_wq, d_wk, d_wv, d_wout,
              d_bout, d_kw1, d_kb1, d_kw2, d_kb2, d_out)
    return nc


def _unit(ap):
    return bass.AP(tensor=ap.tensor, offset=ap.offset, ap=list(ap.ap) + [[1, 1]])


def _col(ap1d):
    return bass.AP(tensor=ap1d.tensor, offset=ap1d.offset,
                   ap=[list(ap1d.ap[0]), [1, 1]])


def _bcast(ap_row, parts):
    # ap_row: [1, N]-ish AP -> broadcast over `parts` partitions via step-0
    return bass.AP(tensor=ap_row.tensor, offset=ap_row.offset,
                   ap=[[0, parts]] + list(ap_row.ap[1:]))


def _body(tc, d_xq, d_xk, d_xv, d_tq, d_tk, d_wq, d_wk, d_wv, d_wout,
          d_bout, d_kw1, d_kb1, d_kw2, d_kb2, d_out):
    nc = tc.nc
    import contextlib
    ctx = contextlib.ExitStack()
    persist = ctx.enter_context(tc.tile_pool(name="persist", bufs=1))
    work = ctx.enter_context(tc.tile_pool(name="work", bufs=3))
    ctx0 = contextlib.ExitStack()
    stage = ctx0.enter_context(tc.tile_pool(name="stage", bufs=2))
    xw_pool = ctx0.enter_context(tc.tile_pool(name="xw", bufs=1))
    psum_w = ctx0.enter_context(tc.tile_pool(name="psum_w", bufs=2, space="PSUM"))
    psum_a = ctx0.enter_context(tc.tile_pool(name="psum_a", bufs=1, space="PSUM"))

    # ================= phase 0: constants & small precompute =================
    # tqT/tkT [2, 1024] via swapped-AP DMA (tiny)
    tqT = persist.tile([DT, NQ], F32)
    tkT = persist.tile([DT, NKV], F32)
    nc.sync.dma_start(out=tqT[:, :], in_=_unit(d_tq.rearrange("q t -> t q")))
    nc.sync.dma_start(out=tkT[:, :], in_=_unit(d_tk.rearrange("k t -> t k")))
    kw1_sb = persist.tile([DT, KHID], F32)
    nc.sync.dma_start(out=kw1_sb[:, :], in_=d_kw1[:, :])
    kb1_sb = persist.tile([KHID, 1], F32)
    nc.sync.dma_start(out=kb1_sb[:, :], in_=_col(d_kb1))
    kw2_sb = persist.tile([KHID, H], F32)
    nc.sync.dma_start(out=kw2_sb[:, :], in_=d_kw2[:, :])
    kb2_sb = persist.tile([H, 1], F32)
    nc.sync.dma_start(out=kb2_sb[:, :], in_=_col(d_kb2))
    bout_bc = persist.tile([P, DX], F32)
    nc.sync.dma_start(
        out=bout_bc[:, :],
        in_=bass.AP(tensor=d_bout.tensor, offset=d_bout.offset,
                    ap=[[0, P], [1, DX]]))

    tqT_bf = persist.tile([DT, NQ], BF16)
    tkT_bf = persist.tile([DT, NKV], BF16)
    kw1_bf = persist.tile([DT, KHID], BF16)
    nc.vector.tensor_copy(tqT_bf[:, :], tqT[:, :])
    nc.vector.tensor_copy(tkT_bf[:, :], tkT[:, :])
    nc.vector.tensor_copy(kw1_bf[:, :], kw1_sb[:, :])
    kw2_bf = persist.tile([KHID, H], BF16)
    nc.vector.tensor_copy(kw2_bf[:, :], kw2_sb[:, :])

    # kb2 pattern tile: partition (half,h,klo) -> kb2[h]  (DRAM step-0 bcast)
    kb2_pat = persist.tile([P, 1], F32)
    for half in range(2):
        for h in range(H):
            nc.sync.dma_start(
                out=kb2_pat[half * 64 + h * 8: half * 64 + h * 8 + 8, :],
                in_=bass.AP(tensor=d_kb2.tensor, offset=d_kb2.offset + h,
                            ap=[[0, 8], [1, 1]]))

    # E matrix [128=(klo,c), 64=(h,klo)]: E[klo*16+c, h*8+klo] = kw2[c,h]
    E_sb = persist.tile([P, 64], BF16)
    nc.vector.memset(E_sb[:, :], 0.0)
    for klo in range(8):
        nc.sync.dma_start(
            out=_unit(E_sb[klo * 16: klo * 16 + 16, klo::8]),
            in_=_unit(kw2_bf[:, :]))

    ones_sb = persist.tile([P, 1], BF16)
    nc.vector.memset(ones_sb[:, :], 1.0)
    zeros_sb = persist.tile([P, P], BF16)
    nc.vector.memset(zeros_sb[:, :], 0.0)

    # aT[c,q] = kw1^T tqT + kb1 (bf16); b[k,c] = -(tk kw1) (f32, per k-tile)
    aT_ps = psum_a.tile([KHID, NQ], F32)
    for j in range(NQ // 512):
        nc.tensor.matmul(aT_ps[:, j * 512:(j + 1) * 512], kw1_bf[:, :],
                         tqT_bf[:, j * 512:(j + 1) * 512], start=True, stop=True)
    aT_bf = persist.tile([KHID, NQ], BF16)
    nc.scalar.activation(aT_bf[:, :], aT_ps[:, :],
                         mybir.ActivationFunctionType.Identity,
                         bias=kb1_sb[:, :], scale=1.0)
    dram_pool = ctx.enter_context(tc.tile_pool(name="drsc", bufs=1, space="DRAM"))
    aT_dram = dram_pool.tile([KHID, NQ], BF16)
    nc.sync.dma_start(out=aT_dram[:, :], in_=aT_bf[:, :])

    b_sb = persist.tile([P, NKT, KHID], F32)
    for kt in range(NKT):
        b_ps = psum_w.tile([P, KHID], F32, tag="b_ps")
        nc.tensor.matmul(b_ps[:, :],
                         tkT_bf[:, kt * P:(kt + 1) * P], kw1_bf[:, :],
                         start=True, stop=True)
        nc.scalar.activation(b_sb[:, kt, :], b_ps[:, :],
                             mybir.ActivationFunctionType.Copy, scale=-1.0)

    # ================= phase 1: projections =================
    # load weights, cast bf16
    def load_w(dram, name):
        w_f = stage.tile([P, NDXT, INNER], F32, tag="w_f")
        nc.sync.dma_start(out=w_f[:, :, :],
                          in_=dram.rearrange("(t p) i -> p t i", p=P))
        w_b = (persist if name == "wout_bf" else xw_pool).tile(
            [P, NDXT, INNER], BF16, tag=name)
        for t in range(NDXT):
            nc.vector.tensor_copy(w_b[:, t, :], w_f[:, t, :])
        return w_b

    wq_bf = load_w(d_wq, "wq_bf")
    wk_bf = load_w(d_wk, "wk_bf")
    wv_bf = load_w(d_wv, "wv_bf")
    wout_bf = load_w(d_wout, "wout_bf")

    # load x straight, cast bf16, DMA-transpose to xT_bf [128, 4, 1024]
    def load_xT(dram, name):
        xT = xw_pool.tile([P, NDXT, NQ], BF16, tag=name)
        for qt in range(NQ // P):
            x_f = stage.tile([P, DX], F32, tag="x_f")
            nc.sync.dma_start(out=x_f[:, :], in_=dram[qt * P:(qt + 1) * P, :])
            x_b = stage.tile([P, DX], BF16, tag="x_b")
            nc.vector.tensor_copy(x_b[:, :], x_f[:, :])
            for dt_ in range(NDXT):
                nc.sync.dma_start_transpose(
                    out=xT[:, dt_, qt * P:(qt + 1) * P],
                    in_=x_b[:, dt_ * P:(dt_ + 1) * P])
        return xT

    xqT_bf = load_xT(d_xq, "xqT_bf")
    xkT_bf = load_xT(d_xk, "xkT_bf")
    xvT_bf = load_xT(d_xv, "xvT_bf")

    # qT/kT [128, 4, 1024] bf16 (qT folded with SCALE); v [128, 8, 512] bf16
    qT_bf = persist.tile([P, NIT, NQ], BF16)
    kT_bf = persist.tile([P, NIT, NKV], BF16)
    v_bf = [persist.tile([P, INNER], BF16, tag=f"v_bf{kt}", name=f"v_bf{kt}")
            for kt in range(NKT)]
    for it in range(NIT):
        for j in range(NQ // 512):
            pq = psum_w.tile([P, 512], F32, tag="proj_ps")
            pk = psum_w.tile([P, 512], F32, tag="proj_ps")
            for dt_ in range(NDXT):
                nc.tensor.matmul(pq[:, :], wq_bf[:, dt_, it * P:(it + 1) * P],
                                 xqT_bf[:, dt_, j * 512:(j + 1) * 512],
                                 start=(dt_ == 0), stop=(dt_ == NDXT - 1))
            for dt_ in range(NDXT):
                nc.tensor.matmul(pk[:, :], wk_bf[:, dt_, it * P:(it + 1) * P],
                                 xkT_bf[:, dt_, j * 512:(j + 1) * 512],
                                 start=(dt_ == 0), stop=(dt_ == NDXT - 1))
            nc.scalar.activation(qT_bf[:, it, j * 512:(j + 1) * 512], pq[:, :],
                                 mybir.ActivationFunctionType.Copy, scale=SCALE)
            nc.scalar.activation(kT_bf[:, it, j * 512:(j + 1) * 512], pk[:, :],
                                 mybir.ActivationFunctionType.Copy, scale=1.0)
    for kt in range(NKT):
        pv = psum_w.tile([P, INNER], F32, tag="proj_ps")
        for dt_ in range(NDXT):
            nc.tensor.matmul(pv[:, :], xvT_bf[:, dt_, kt * P:(kt + 1) * P],
                             wv_bf[:, dt_, :],
                             start=(dt_ == 0), stop=(dt_ == NDXT - 1))
        nc.vector.tensor_copy(v_bf[kt][:, :], pv[:, :])

    # ================= phase 2: attention per q-chunk =================
    ctx0.close()   # free phase-0/1 transient SBUF + PSUM pools
    ctx2 = contextlib.ExitStack()
    psum_acc = ctx2.enter_context(tc.tile_pool(name="psum_acc", bufs=1, space="PSUM"))
    rshuf_pool = ctx2.enter_context(tc.tile_pool(name="rshuf", bufs=2))
    rk_pool = ctx2.enter_context(tc.tile_pool(name="rk", bufs=2))
    eb_pool = ctx2.enter_context(tc.tile_pool(name="eb", bufs=2))
    ebs_pool = ctx2.enter_context(tc.tile_pool(name="ebs", bufs=2))
    et_pool = ctx2.enter_context(tc.tile_pool(name="et", bufs=4))
    p_pool = ctx2.enter_context(tc.tile_pool(name="pp", bufs=4))
    abc_pool = ctx2.enter_context(tc.tile_pool(name="abc", bufs=1))
    psum_d = ctx2.enter_context(tc.tile_pool(name="psum_d", bufs=1, space="PSUM"))

    outT_sb = persist.tile([P, NIT, NQ], BF16)   # [ (h-pair d), it, q ]

    for qc in range(NQC):
        q0 = qc * QC
        # A_bc [128, c, 512] bf16 broadcast of aT
        A_bc = abc_pool.tile([P, KHID, QC], BF16)
        for c in range(KHID):
            nc.sync.dma_start(
                out=A_bc[:, c, :],
                in_=bass.AP(tensor=aT_dram.tensor,
                            offset=aT_dram[c:c + 1, q0:q0 + QC].offset,
                            ap=[[0, P], [1, QC]]))

        # persistent accumulators for this q-chunk
        av_ps = psum_acc.tile([P, NIT, QC], F32, tag="av")      # 4 banks, 2 heads each
        z_ps = psum_acc.tile([P, 2, QC], F32, tag="z")          # 2 banks, 4 slots each
        for it in range(NIT):
            nc.tensor.matmul(av_ps[:, it, :], zeros_sb[:, :],
                             wout_bf[:, 0, :], start=True, stop=False)
        for zb in range(2):
            nc.tensor.matmul(z_ps[:, zb, :], zeros_sb[:, :],
                             wout_bf[:, 0, :], start=True, stop=False)

        for kt in range(NKT):
            # ---- R production: R_k[k, c, q] = relu(A_bc + b) ----
            R_k = rk_pool.tile([P, KHID, QC], BF16)
            for c in range(KHID):
                nc.vector.tensor_scalar(
                    out=R_k[:, c, :], in0=A_bc[:, c, :],
                    scalar1=b_sb[:, kt, c:c + 1], scalar2=0.0,
                    op0=AX.add, op1=AX.max)
            # ---- R shuffle: -> R_shuf[(klo,c), kgrp, q] ----
            R_shuf = rshuf_pool.tile([P, 16, QC], BF16)
            for kg in range(16):
                nc.sync.dma_start(
                    out=R_shuf[:, kg, :],
                    in_=R_k[kg * 8:(kg + 1) * 8, :, :])
            # ---- E-mm pairs + exp -> E_b_sb [128=(half,h,klo), pair, q] ----
            E_b = eb_pool.tile([P, 8, QC], BF16)
            for pair in range(8):
                bias_ps = psum_d.tile([P, QC], F32, tag="bias_ps")
                nc.tensor.matmul(bias_ps[0:64, :], E_sb[:, :],
                                 R_shuf[:, pair, :], start=True, stop=True)
                nc.tensor.matmul(bias_ps[64:128, :], E_sb[:, :],
                                 R_shuf[:, 8 + pair, :], start=True, stop=True,
                                 tile_position=(0, 64))
                nc.scalar.activation(E_b[:, pair, :], bias_ps[:, :],
                                     mybir.ActivationFunctionType.Exp,
                                     bias=kb2_pat[:, :], scale=1.0)
            # ---- E_b shuffle -> E_b_shuf [k, h, q] ----
            E_b_shuf = ebs_pool.tile([P, H, QC], BF16)
            for h in range(H):
                for half in range(2):
                    for pair in range(8):
                        nc.sync.dma_start(
                            out=E_b_shuf[half * 64 + pair * 8:
                                         half * 64 + pair * 8 + 8, h, :],
                            in_=E_b[half * 64 + h * 8: half * 64 + h * 8 + 8,
                                    pair, :])
            # ---- per head: QK, exp, P=Et*Eb, AV, Z ----
            for h in range(H):
                it = h // 2
                r0 = (h % 2) * 64
                dots_ps = psum_d.tile([P, QC], F32, tag="dots_ps")
                nc.tensor.matmul(
                    dots_ps[:, :],
                    kT_bf[r0:r0 + 64, it, kt * P:(kt + 1) * P],
                    qT_bf[r0:r0 + 64, it, q0:q0 + QC],
                    start=True, stop=True)
                E_t = et_pool.tile([P, QC], BF16)
                nc.scalar.activation(E_t[:, :], dots_ps[:, :],
                                     mybir.ActivationFunctionType.Exp)
                P_sb = p_pool.tile([P, QC], BF16)
                nc.vector.tensor_mul(P_sb[:, :], E_t[:, :], E_b_shuf[:, h, :])
                nc.tensor.matmul(
                    av_ps[r0:r0 + 64, it, :],
                    v_bf[kt][:, h * 64:(h + 1) * 64], P_sb[:, :],
                    start=False, stop=False,
                    tile_position=(0, r0) if r0 else None)
                zslot = h % 4
                nc.tensor.matmul(
                    z_ps[zslot * 32: zslot * 32 + 1, h // 4, :],
                    ones_sb[:, :], P_sb[:, :],
                    start=False, stop=False,
                    tile_position=(0, zslot * 32) if zslot else None)

        for it in range(NIT):
            nc.tensor.matmul(av_ps[:, it, :], zeros_sb[:, :],
                             wout_bf[:, 0, :], start=False, stop=True)
        for zb in range(2):
            nc.tensor.matmul(z_ps[:, zb, :], zeros_sb[:, :],
                             wout_bf[:, 0, :], start=False, stop=True)
        # ---- normalize: outT = av / Z (recip -> DRAM bounce -> bcast) ----
        zr_dram = dram_pool.tile([H, QC], F32, tag="zr_dram")
        for h in range(H):
            zr_t = work.tile([1, QC], F32, tag="zr_t")
            nc.vector.reciprocal(zr_t[:, :],
                                 z_ps[(h % 4) * 32:(h % 4) * 32 + 1, h // 4, :])
            nc.sync.dma_start(out=zr_dram[h:h + 1, :], in_=zr_t[:, :])
        for h in range(H):
            zr_bc = work.tile([64, QC], F32, tag="zr_bc")
            nc.sync.dma_start(
                out=zr_bc[:, :],
                in_=bass.AP(tensor=zr_dram.tensor,
                            offset=zr_dram[h:h + 1, :].offset,
                            ap=[[0, 64], [1, QC]]))
            r0 = (h % 2) * 64
            nc.vector.tensor_mul(outT_sb[r0:r0 + 64, h // 2, q0:q0 + QC],
                                 av_ps[r0:r0 + 64, h // 2, :], zr_bc[:, :])

    # ================= phase 3: output projection =================
    ctx2.close()   # free phase-2 pools
    psum_o = ctx.enter_context(tc.tile_pool(name="psum_o", bufs=2, space="PSUM"))
    for qt in range(NQ // P):
        op = psum_o.tile([P, DX], F32, tag="op")
        for it in range(NIT):
            nc.tensor.matmul(op[:, :],
                             outT_sb[:, it, qt * P:(qt + 1) * P],
                             wout_bf[:, it, :],
                             start=(it == 0), stop=(it == NIT - 1))
        o_sb = work.tile([P, DX], F32, tag="o_sb")
        nc.vector.tensor_add(o_sb[:, :], op[:, :], bout_bc[:, :])
        nc.sync.dma_start(out=d_out[qt * P:(qt + 1) * P, :], in_=o_sb[:, :])

    ctx.close()


_NC_CACHE = None


def _get_nc():
    global _NC_CACHE
    if _NC_CACHE is None:
        nc = bacc.Bacc("TRN2", target_bir_lowering=False, debug=False,
                       enable_asserts=False, num_devices=M)
        build_kernel(nc)
        nc.compile()
        _NC_CACHE = nc
    return _NC_CACHE


def kernel(**inputs):
    nc = _get_nc()
    shared = {n: np.ascontiguousarray(np.asarray(inputs[n], dtype=np.float32))
              for n in ["w_q", "w_k", "w_v", "w_out", "b_out",
                        "kw1", "kb1", "kw2", "kb2"]}
    in_maps = []
    for i in range(M):
        m = dict(shared)
        for n in ["xq", "xk", "xv", "tq", "tk"]:
            m[n] = np.ascontiguousarray(np.asarray(inputs[n][i], dtype=np.float32))
        in_maps.append(m)
    res = bass_utils.run_bass_kernel_spmd(nc, in_maps, core_ids=list(range(M)))
    out = np.stack([res.results[i]["out"] for i in range(M)], axis=0)
    return out.astype(np.float32)


if __name__ == "__main__":
    import reference
    inputs = {k: np.asarray(v) for k, v in reference.setup_inputs().items()}
    out = kernel(**inputs)
    print("out", out.shape, out.dtype)


# revision 29
# speedup vs baseline: 1042.8917x; 1.0000x over previous
"""MultiHeadTEAttention TRN2 kernel — 8-core SPMD, one batch element per core.

Architecture (per core, batch m):
  - Transposed-K ("flash") layout: dots^T[k,q] per head; softmax over the
    partition dim via PE ones-matmul column sums (values bounded, no max pass).
  - Kernel-MLP bias: R[k,(c,q)] = relu(b[k,c] + a[q,c]) via DVE tensor_scalar
    (fused add+relu, bf16); DMA partition-shuffle to [(k_lo,c),(q)] layout;
    PE contraction with a fixed E-matrix E[(k_lo,c),(h,k_lo)] = kw2[c,h];
    exp on ACT straight out of PSUM; DMA partition-shuffle back to per-head
    [k,q]; multiplied into exp(token_dots) on DVE.
  - AV and token-dots contract on PE in bf16; fp32 PSUM accumulation.
"""

import numpy as np

import concourse.bass as bass
import concourse.mybir as mybir
import concourse.tile as tile
from concourse import bacc, bass_utils

F32 = mybir.dt.float32
BF16 = mybir.dt.bfloat16
AX = mybir.AluOpType

M, NQ, NKV, DX, DT = 8, 1024, 1024, 512, 2
H, HD = 8, 64
INNER = H * HD          # 512
KHID = 16               # c
SCALE = HD ** -0.5
P = 128
NKT = NKV // P          # 8 k-tiles
NQC = 2                 # q chunks of 512
QC = NQ // NQC          # 512
NIT = INNER // P        # 4 inner tiles
NDXT = DX // P          # 4 dx tiles


def build_kernel(nc: bass.Bass):
    # ---- DRAM I/O ----
    d_xq = nc.dram_tensor("xq", [NQ, DX], F32, kind="ExternalInput").ap()
    d_xk = nc.dram_tensor("xk", [NKV, DX], F32, kind="ExternalInput").ap()
    d_xv = nc.dram_tensor("xv", [NKV, DX], F32, kind="ExternalInput").ap()
    d_tq = nc.dram_tensor("tq", [NQ, DT], F32, kind="ExternalInput").ap()
    d_tk = nc.dram_tensor("tk", [NKV, DT], F32, kind="ExternalInput").ap()
    d_wq = nc.dram_tensor("w_q", [DX, INNER], F32, kind="ExternalInput").ap()
    d_wk = nc.dram_tensor("w_k", [DX, INNER], F32, kind="ExternalInput").ap()
    d_wv = nc.dram_tensor("w_v", [DX, INNER], F32, kind="ExternalInput").ap()
    d_wout = nc.dram_tensor("w_out", [INNER, DX], F32, kind="ExternalInput").ap()
    d_bout = nc.dram_tensor("b_out", [DX], F32, kind="ExternalInput").ap()
    d_kw1 = nc.dram_tensor("kw1", [DT, KHID], F32, kind="ExternalInput").ap()
    d_kb1 = nc.dram_tensor("kb1", [KHID], F32, kind="ExternalInput").ap()
    d_kw2 = nc.dram_tensor("kw2", [KHID, H], F32, kind="ExternalInput").ap()
    d_kb2 = nc.dram_tensor("kb2", [H], F32, kind="ExternalInput").ap()
    d_out = nc.dram_tensor("out", [NQ, DX], F32, kind="ExternalOutput").ap()

    with tile.TileContext(nc) as tc:
        _body(tc, d_xq, d_xk, d_xv, d_tq, d_tk, d_wq, d_wk, d_wv, d_wout,
              d_bout, d_kw1, d_kb1, d_kw2, d_kb2, d_out)
    return nc


def _unit(ap):
    return bass.AP(tensor=ap.tensor, offset=ap.offset, ap=list(ap.ap) + [[1, 1]])


def _col(ap1d):
    return bass.AP(tensor=ap1d.tensor, offset=ap1d.offset,
                   ap=[list(ap1d.ap[0]), [1, 1]])


def _bcast(ap_row, parts):
    # ap_row: [1, N]-ish AP -> broadcast over `parts` partitions via step-0
    return bass.AP(tensor=ap_row.tensor, offset=ap_row.offset,
                   ap=[[0, parts]] + list(ap_row.ap[1:]))


def _body(tc, d_xq, d_xk, d_xv, d_tq, d_tk, d_wq, d_wk, d_wv, d_wout,
          d_bout, d_kw1, d_kb1, d_kw2, d_kb2, d_out):
    nc = tc.nc
    import contextlib
    ctx = contextlib.ExitStack()
    persist = ctx.enter_context(tc.tile_pool(name="persist", bufs=1))
    work = ctx.enter_context(tc.tile_pool(name="work", bufs=3))
    ctx0 = contextlib.ExitStack()
    stage = ctx0.enter_context(tc.tile_pool(name="stage", bufs=2))
    xw_pool = ctx0.enter_context(tc.tile_pool(name="xw", bufs=1))
    psum_w = ctx0.enter_context(tc.tile_pool(name="psum_w", bufs=2, space="PSUM"))
    psum_a = ctx0.enter_context(tc.tile_pool(name="psum_a", bufs=1, space="PSUM"))

    # ================= phase 0: constants & small precompute =================
    # tqT/tkT [2, 1024] via swapped-AP DMA (tiny)
    tqT = persist.tile([DT, NQ], F32)
    tkT = persist.tile([DT, NKV], F32)
    nc.sync.dma_start(out=tqT[:, :], in_=_unit(d_tq.rearrange("q t -> t q")))
    nc.sync.dma_start(out=tkT[:, :], in_=_unit(d_tk.rearrange("k t -> t k")))
    kw1_sb = persist.tile([DT, KHID], F32)
    nc.sync.dma_start(out=kw1_sb[:, :], in_=d_kw1[:, :])
    kb1_sb = persist.tile([KHID, 1], F32)
    nc.sync.dma_start(out=kb1_sb[:, :], in_=_col(d_kb1))
    kw2_sb = persist.tile([KHID, H], F32)
    nc.sync.dma_start(out=kw2_sb[:, :], in_=d_kw2[:, :])
    kb2_sb = persist.tile([H, 1], F32)
    nc.sync.dma_start(out=kb2_sb[:, :], in_=_col(d_kb2))
    bout_bc = persist.tile([P, DX], F32)
    nc.sync.dma_start(
        out=bout_bc[:, :],
        in_=bass.AP(tensor=d_bout.tensor, offset=d_bout.offset,
                    ap=[[0, P], [1, DX]]))

    tqT_bf = persist.tile([DT, NQ], BF16)
    tkT_bf = persist.tile([DT, NKV], BF16)
    kw1_bf = persist.tile([DT, KHID], BF16)
    nc.vector.tensor_copy(tqT_bf[:, :], tqT[:, :])
    nc.vector.tensor_copy(tkT_bf[:, :], tkT[:, :])
    nc.vector.tensor_copy(kw1_bf[:, :], kw1_sb[:, :])
    kw2_bf = persist.tile([KHID, H], BF16)
    nc.vector.tensor_copy(kw2_bf[:, :], kw2_sb[:, :])

    # kb2 pattern tile: partition (half,h,klo) -> kb2[h]  (DRAM step-0 bcast)
    kb2_pat = persist.tile([P, 1], F32)
    for half in range(2):
        for h in range(H):
            nc.sync.dma_start(
                out=kb2_pat[half * 64 + h * 8: half * 64 + h * 8 + 8, :],
                in_=bass.AP(tensor=d_kb2.tensor, offset=d_kb2.offset + h,
                            ap=[[0, 8], [1, 1]]))

    # E matrix [128=(klo,c), 64=(h,klo)]: E[klo*16+c, h*8+klo] = kw2[c,h]
    E_sb = persist.tile([P, 64], BF16)
    nc.vector.memset(E_sb[:, :], 0.0)
    for klo in range(8):
        nc.sync.dma_start(
            out=_unit(E_sb[klo * 16: klo * 16 + 16, klo::8]),
            in_=_unit(kw2_bf[:, :]))

    ones_sb = persist.tile([P, 1], BF16)
    nc.vector.memset(ones_sb[:, :], 1.0)
    zeros_sb = persist.tile([P, P], BF16)
    nc.vector.memset(zeros_sb[:, :], 0.0)

    # aT[c,q] = kw1^T tqT + kb1 (bf16); b[k,c] = -(tk kw1) (f32, per k-tile)
    aT_ps = psum_a.tile([KHID, NQ], F32)
    for j in range(NQ // 512):
        nc.tensor.matmul(aT_ps[:, j * 512:(j + 1) * 512], kw1_bf[:, :],
                         tqT_bf[:, j * 512:(j + 1) * 512], start=True, stop=True)
    aT_bf = persist.tile([KHID, NQ], BF16)
    nc.scalar.activation(aT_bf[:, :], aT_ps[:, :],
                         mybir.ActivationFunctionType.Identity,
                         bias=kb1_sb[:, :], scale=1.0)
    dram_pool = ctx.enter_context(tc.tile_pool(name="drsc", bufs=1, space="DRAM"))
    aT_dram = dram_pool.tile([KHID, NQ], BF16)
    nc.sync.dma_start(out=aT_dram[:, :], in_=aT_bf[:, :])

    b_sb = persist.tile([P, NKT, KHID], F32)
    for kt in range(NKT):
        b_ps = psum_w.tile([P, KHID], F32, tag="b_ps")
        nc.tensor.matmul(b_ps[:, :], tkT_bf[:, kt * P:(kt + 1) * P],
                         kw1_bf[:, :], start=True, stop=True)
        nc.scalar.activation(b_sb[:, kt, :], b_ps[:, :],
                             mybir.ActivationFunctionType.Copy, scale=-1.0)

    # ================= phase 1: projections =================
    # load weights, cast bf16
    def load_w(dram, name):
        w_f = stage.tile([P, NDXT, INNER], F32, tag="w_f")
        nc.sync.dma_start(out=w_f[:, :, :],
                          in_=dram.rearrange("(t p) i -> p t i", p=P))
        w_b = (persist if name == "wout_bf" else xw_pool).tile(
            [P, NDXT, INNER], BF16, tag=name)
        for t in range(NDXT):
            nc.vector.tensor_copy(w_b[:, t, :], w_f[:, t, :])
        return w_b

    wq_bf = load_w(d_wq, "wq_bf")
    wk_bf = load_w(d_wk, "wk_bf")
    wv_bf = load_w(d_wv, "wv_bf")
    wout_bf = load_w(d_wout, "wout_bf")

    # load x straight, cast bf16, DMA-transpose to xT_bf [128, 4, 1024]
    def load_xT(dram, name):
        xT = xw_pool.tile([P, NDXT, NQ], BF16, tag=name)
        for qt in range(NQ // P):
            x_f = stage.tile([P, DX], F32, tag="x_f")
            nc.sync.dma_start(out=x_f[:, :], in_=dram[qt * P:(qt + 1) * P, :])
            x_b = stage.tile([P, DX], BF16, tag="x_b")
            nc.vector.tensor_copy(x_b[:, :], x_f[:, :])
            for dt_ in range(NDXT):
                nc.sync.dma_start_transpose(
                    out=xT[:, dt_, qt * P:(qt + 1) * P],
                    in_=x_b[:, dt_ * P:(dt_ + 1) * P])
        return xT

    xqT_bf = load_xT(d_xq, "xqT_bf")
    xkT_bf = load_xT(d_xk, "xkT_bf")
    xvT_bf = load_xT(d_xv, "xvT_bf")

    # qT/kT [128, 4, 1024] bf16 (qT folded with SCALE); v [128, 8, 512] bf16
    qT_bf = persist.tile([P, NIT, NQ], BF16)
    kT_bf = persist.tile([P, NIT, NKV], BF16)
    v_bf = [persist.tile([P, INNER], BF16, tag=f"v_bf{kt}", name=f"v_bf{kt}")
            for kt in range(NKT)]
    for it in range(NIT):
        for j in range(NQ // 512):
            pq = psum_w.tile([P, 512], F32, tag="proj_ps")
            pk = psum_w.tile([P, 512], F32, tag="proj_ps")
            for dt_ in range(NDXT):
                nc.tensor.matmul(pq[:, :], wq_bf[:, dt_, it * P:(it + 1) * P],
                                 xqT_bf[:, dt_, j * 512:(j + 1) * 512],
                                 start=(dt_ == 0), stop=(dt_ == NDXT - 1))
            for dt_ in range(NDXT):
                nc.tensor.matmul(pk[:, :], wk_bf[:, dt_, it * P:(it + 1) * P],
                                 xkT_bf[:, dt_, j * 512:(j + 1) * 512],
                                 start=(dt_ == 0), stop=(dt_ == NDXT - 1))
            nc.scalar.activation(qT_bf[:, it, j * 512:(j + 1) * 512], pq[:, :],
                                 mybir.ActivationFunctionType.Copy, scale=SCALE)
            nc.scalar.activation(kT_bf[:, it, j * 512:(j + 1) * 512], pk[:, :],
                                 mybir.ActivationFunctionType.Copy, scale=1.0)
    for kt in range(NKT):
        pv = psum_w.tile([P, INNER], F32, tag="proj_ps")
        for dt_ in range(NDXT):
            nc.tensor.matmul(pv[:, :], xvT_bf[:, dt_, kt * P:(kt + 1) * P],
                             wv_bf[:, dt_, :],
                             start=(dt_ == 0), stop=(dt_ == NDXT - 1))
        nc.vector.tensor_copy(v_bf[kt][:, :], pv[:, :])

    # ================= phase 2: attention per q-chunk =================
    ctx0.close()   # free phase-0/1 transient SBUF + PSUM pools
    ctx2 = contextlib.ExitStack()
    psum_acc = ctx2.enter_context(tc.tile_pool(name="psum_acc", bufs=1, space="PSUM"))
    rshuf_pool = ctx2.enter_context(tc.tile_pool(name="rshuf", bufs=2))
    rk_pool = ctx2.enter_context(tc.tile_pool(name="rk", bufs=2))
    eb_pool = ctx2.enter_context(tc.tile_pool(name="eb", bufs=2))
    ebs_pool = ctx2.enter_context(tc.tile_pool(name="ebs", bufs=2))
    et_pool = ctx2.enter_context(tc.tile_pool(name="et", bufs=4))
    p_pool = ctx2.enter_context(tc.tile_pool(name="pp", bufs=4))
    abc_pool = ctx2.enter_context(tc.tile_pool(name="abc", bufs=1))
    psum_d = ctx2.enter_context(tc.tile_pool(name="psum_d", bufs=1, space="PSUM"))

    outT_sb = persist.tile([P, NIT, NQ], BF16)   # [ (h-pair d), it, q ]

    for qc in range(NQC):
        q0 = qc * QC
        # A_bc [128, c, 512] bf16 broadcast of aT
        A_bc = abc_pool.tile([P, KHID, QC], BF16)
        for c in range(KHID):
            nc.sync.dma_start(
                out=A_bc[:, c, :],
                in_=bass.AP(tensor=aT_dram.tensor,
                            offset=aT_dram[c:c + 1, q0:q0 + QC].offset,
                            ap=[[0, P], [1, QC]]))

        # persistent accumulators for this q-chunk
        av_ps = psum_acc.tile([P, NIT, QC], F32, tag="av")      # 4 banks, 2 heads each
        z_ps = psum_acc.tile([P, 2, QC], F32, tag="z")          # 2 banks, 4 slots each
        for it in range(NIT):
            nc.tensor.matmul(av_ps[:, it, :], zeros_sb[:, :],
                             wout_bf[:, 0, :], start=True, stop=False)
        for zb in range(2):
            nc.tensor.matmul(z_ps[:, zb, :], zeros_sb[:, :],
                             wout_bf[:, 0, :], start=True, stop=False)

        for kt in range(NKT):
            # ---- R production: R_k[k, c, q] = relu(A_bc + b) ----
            R_k = rk_pool.tile([P, KHID, QC], BF16)
            for c in range(KHID):
                nc.vector.tensor_scalar(
                    out=R_k[:, c, :], in0=A_bc[:, c, :],
                    scalar1=b_sb[:, kt, c:c + 1], scalar2=0.0,
                    op0=AX.add, op1=AX.max)
            # ---- R shuffle: -> R_shuf[(klo,c), kgrp, q] ----
            R_shuf = rshuf_pool.tile([P, 16, QC], BF16)
            for kg in range(16):
                nc.sync.dma_start(
                    out=R_shuf[:, kg, :],
                    in_=R_k[kg * 8:(kg + 1) * 8, :, :])
            # ---- E-mm pairs + exp -> E_b_sb [128=(half,h,klo), pair, q] ----
            E_b = eb_pool.tile([P, 8, QC], BF16)
            for pair in range(8):
                bias_ps = psum_d.tile([P, QC], F32, tag="bias_ps")
                nc.tensor.matmul(bias_ps[0:64, :], E_sb[:, :],
                                 R_shuf[:, 2 * pair, :], start=True, stop=True)
                nc.tensor.matmul(bias_ps[64:128, :], E_sb[:, :],
                                 R_shuf[:, 2 * pair + 1, :], start=True, stop=True,
                                 tile_position=(0, 64))
                nc.scalar.activation(E_b[:, pair, :], bias_ps[:, :],
                                     mybir.ActivationFunctionType.Exp,
                                     bias=kb2_pat[:, :], scale=1.0)
            # ---- E_b shuffle -> E_b_shuf [k, h, q] ----
            E_b_shuf = ebs_pool.tile([P, H, QC], BF16)
            for h in range(H):
                for pair in range(8):
                    for half in range(2):
                        nc.sync.dma_start(
                            out=E_b_shuf[pair * 16 + half * 8:
                                         pair * 16 + half * 8 + 8, h, :],
                            in_=E_b[half * 64 + h * 8: half * 64 + h * 8 + 8,
                                    pair, :])
            # ---- per head: QK, exp, P=Et*Eb, AV, Z ----
            for h in range(H):
                it = h // 2
                r0 = (h % 2) * 64
                dots_ps = psum_d.tile([P, QC], F32, tag="dots_ps")
                nc.tensor.matmul(
                    dots_ps[:, :],
                    kT_bf[r0:r0 + 64, it, kt * P:(kt + 1) * P],
                    qT_bf[r0:r0 + 64, it, q0:q0 + QC],
                    start=True, stop=True)
                E_t = et_pool.tile([P, QC], BF16)
                nc.scalar.activation(E_t[:, :], dots_ps[:, :],
                                     mybir.ActivationFunctionType.Exp)
                P_sb = p_pool.tile([P, QC], BF16)
                nc.vector.tensor_mul(P_sb[:, :], E_t[:, :], E_b_shuf[:, h, :])
                nc.tensor.matmul(
                    av_ps[r0:r0 + 64, it, :],
                    v_bf[kt][:, h * 64:(h + 1) * 64], P_sb[:, :],
                    start=False, stop=False,
                    tile_position=(0, r0) if r0 else None)
                zslot = h % 4
                nc.tensor.matmul(
                    z_ps[zslot * 32: zslot * 32 + 1, h // 4, :],
                    ones_sb[:, :], P_sb[:, :],
                    start=False, stop=False,
                    tile_position=(0, zslot * 32) if zslot else None)

        for it in range(NIT):
            nc.tensor.matmul(av_ps[:, it, :], zeros_sb[:, :],
                             wout_bf[:, 0, :], start=False, stop=True)
        for zb in range(2):
            nc.tensor.matmul(z_ps[:, zb, :], zeros_sb[:, :],
                             wout_bf[:, 0, :], start=False, stop=True)
        # ---- normalize: outT = av / Z (recip -> DRAM bounce -> bcast) ----
        zr_dram = dram_pool.tile([H, QC], F32, tag="zr_dram")
        for h in range(H):
            zr_t = work.tile([1, QC], F32, tag="zr_t")
            nc.vector.reciprocal(zr_t[:, :],
                                 z_ps[(h % 4) * 32:(h % 4) * 32 + 1, h // 4, :])
            nc.sync.dma_start(out=zr_dram[h:h + 1, :], in_=zr_t[:, :])
        for h in range(H):
            zr_bc = work.tile([64, QC], F32, tag="zr_bc")
            nc.sync.dma_start(
                out=zr_bc[:, :],
                in_=bass.AP(tensor=zr_dram.tensor,
                            offset=zr_dram[h:h + 1, :].offset,
                            ap=[[0, 64], [1, QC]]))
            r0 = (h % 2) * 64
            nc.vector.tensor_mul(outT_sb[r0:r0 + 64, h // 2, q0:q0 + QC],
                                 av_ps[r0:r0 + 64, h // 2, :], zr_bc[:, :])

    # ================= phase 3: output projection =================
    ctx2.close()   # free phase-2 pools
    psum_o = ctx.enter_context(tc.tile_pool(name="psum_o", bufs=2, space="PSUM"))
    for qt in range(NQ // P):
        op = psum_o.tile([P, DX], F32, tag="op")
        for it in range(NIT):
            nc.tensor.matmul(op[:, :],
                             outT_sb[:, it, qt * P:(qt + 1) * P],
                             wout_bf[:, it, :],
                             start=(it == 0), stop=(it == NIT - 1))
        o_sb = work.tile([P, DX], F32, tag="o_sb")
        nc.vector.tensor_add(o_sb[:, :], op[:, :], bout_bc[:, :])
        nc.sync.dma_start(out=d_out[qt * P:(qt + 1) * P, :], in_=o_sb[:, :])

    ctx.close()


_NC_CACHE = None


def _get_nc():
    global _NC_CACHE
    if _NC_CACHE is None:
        nc = bacc.Bacc("TRN2", target_bir_lowering=False, debug=False,
                       enable_asserts=False, num_devices=M)
        build_kernel(nc)
        nc.compile()
        _NC_CACHE = nc
    return _NC_CACHE


def kernel(**inputs):
    nc = _get_nc()
    shared = {n: np.ascontiguousarray(np.asarray(inputs[n], dtype=np.float32))
              for n in ["w_q", "w_k", "w_v", "w_out", "b_out",
                        "kw1", "kb1", "kw2", "kb2"]}
    in_maps = []
    for i in range(M):
        m = dict(shared)
        for n in ["xq", "xk", "xv", "tq", "tk"]:
            m[n] = np.ascontiguousarray(np.asarray(inputs[n][i], dtype=np.float32))
        in_maps.append(m)
    res = bass_utils.run_bass_kernel_spmd(nc, in_maps, core_ids=list(range(M)))
    out = np.stack([res.results[i]["out"] for i in range(M)], axis=0)
    return out.astype(np.float32)


if __name__ == "__main__":
    import reference
    inputs = {k: np.asarray(v) for k, v in reference.setup_inputs().items()}
    out = kernel(**inputs)
    print("out", out.shape, out.dtype)


# revision 30
# speedup vs baseline: 1157.3884x; 1.1098x over previous
"""MultiHeadTEAttention TRN2 kernel — 8-core SPMD, one batch element per core.

Architecture (per core, batch m):
  - Transposed-K ("flash") layout: dots^T[k,q] per head; softmax over the
    partition dim via PE ones-matmul column sums (values bounded, no max pass).
  - Kernel-MLP bias: R[k,(c,q)] = relu(b[k,c] + a[q,c]) via DVE tensor_scalar
    (fused add+relu, bf16); DMA partition-shuffle to [(k_lo,c),(q)] layout;
    PE contraction with a fixed E-matrix E[(k_lo,c),(h,k_lo)] = kw2[c,h];
    exp on ACT straight out of PSUM; DMA partition-shuffle back to per-head
    [k,q]; multiplied into exp(token_dots) on DVE.
  - AV and token-dots contract on PE in bf16; fp32 PSUM accumulation.
"""

import numpy as np

import concourse.bass as bass
import concourse.mybir as mybir
import concourse.tile as tile
from concourse import bacc, bass_utils

F32 = mybir.dt.float32
BF16 = mybir.dt.bfloat16
AX = mybir.AluOpType

M, NQ, NKV, DX, DT = 8, 1024, 1024, 512, 2
H, HD = 8, 64
INNER = H * HD          # 512
KHID = 16               # c
SCALE = HD ** -0.5
P = 128
NKT = NKV // P          # 8 k-tiles
NQC = 2                 # q chunks of 512
QC = NQ // NQC          # 512
NIT = INNER // P        # 4 inner tiles
NDXT = DX // P          # 4 dx tiles


def build_kernel(nc: bass.Bass):
    # ---- DRAM I/O ----
    d_xq = nc.dram_tensor("xq", [NQ, DX], F32, kind="ExternalInput").ap()
    d_xk = nc.dram_tensor("xk", [NKV, DX], F32, kind="ExternalInput").ap()
    d_xv = nc.dram_tensor("xv", [NKV, DX], F32, kind="ExternalInput").ap()
    d_tq = nc.dram_tensor("tq", [NQ, DT], F32, kind="ExternalInput").ap()
    d_tk = nc.dram_tensor("tk", [NKV, DT], F32, kind="ExternalInput").ap()
    d_wq = nc.dram_tensor("w_q", [DX, INNER], F32, kind="ExternalInput").ap()
    d_wk = nc.dram_tensor("w_k", [DX, INNER], F32, kind="ExternalInput").ap()
    d_wv = nc.dram_tensor("w_v", [DX, INNER], F32, kind="ExternalInput").ap()
    d_wout = nc.dram_tensor("w_out", [INNER, DX], F32, kind="ExternalInput").ap()
    d_bout = nc.dram_tensor("b_out", [DX], F32, kind="ExternalInput").ap()
    d_kw1 = nc.dram_tensor("kw1", [DT, KHID], F32, kind="ExternalInput").ap()
    d_kb1 = nc.dram_tensor("kb1", [KHID], F32, kind="ExternalInput").ap()
    d_kw2 = nc.dram_tensor("kw2", [KHID, H], F32, kind="ExternalInput").ap()
    d_kb2 = nc.dram_tensor("kb2", [H], F32, kind="ExternalInput").ap()
    d_out = nc.dram_tensor("out", [NQ, DX], F32, kind="ExternalOutput").ap()

    with tile.TileContext(nc) as tc:
        _body(tc, d_xq, d_xk, d_xv, d_tq, d_tk, d_wq, d_wk, d_wv, d_wout,
              d_bout, d_kw1, d_kb1, d_kw2, d_kb2, d_out)
    return nc


def _unit(ap):
    return bass.AP(tensor=ap.tensor, offset=ap.offset, ap=list(ap.ap) + [[1, 1]])


def _col(ap1d):
    return bass.AP(tensor=ap1d.tensor, offset=ap1d.offset,
                   ap=[list(ap1d.ap[0]), [1, 1]])


def _bcast(ap_row, parts):
    # ap_row: [1, N]-ish AP -> broadcast over `parts` partitions via step-0
    return bass.AP(tensor=ap_row.tensor, offset=ap_row.offset,
                   ap=[[0, parts]] + list(ap_row.ap[1:]))


def _body(tc, d_xq, d_xk, d_xv, d_tq, d_tk, d_wq, d_wk, d_wv, d_wout,
          d_bout, d_kw1, d_kb1, d_kw2, d_kb2, d_out):
    nc = tc.nc
    import contextlib
    ctx = contextlib.ExitStack()
    persist = ctx.enter_context(tc.tile_pool(name="persist", bufs=1))
    work = ctx.enter_context(tc.tile_pool(name="work", bufs=3))
    ctx0 = contextlib.ExitStack()
    stage = ctx0.enter_context(tc.tile_pool(name="stage", bufs=2))
    xw_pool = ctx0.enter_context(tc.tile_pool(name="xw", bufs=1))
    psum_w = ctx0.enter_context(tc.tile_pool(name="psum_w", bufs=2, space="PSUM"))
    psum_a = ctx0.enter_context(tc.tile_pool(name="psum_a", bufs=1, space="PSUM"))

    # ================= phase 0: constants & small precompute =================
    # tqT/tkT [2, 1024] via swapped-AP DMA (tiny)
    tqT = persist.tile([DT, NQ], F32)
    tkT = persist.tile([DT, NKV], F32)
    nc.sync.dma_start(out=tqT[:, :], in_=_unit(d_tq.rearrange("q t -> t q")))
    nc.sync.dma_start(out=tkT[:, :], in_=_unit(d_tk.rearrange("k t -> t k")))
    kw1_sb = persist.tile([DT, KHID], F32)
    nc.sync.dma_start(out=kw1_sb[:, :], in_=d_kw1[:, :])
    kb1_sb = persist.tile([KHID, 1], F32)
    nc.sync.dma_start(out=kb1_sb[:, :], in_=_col(d_kb1))
    kw2_sb = persist.tile([KHID, H], F32)
    nc.sync.dma_start(out=kw2_sb[:, :], in_=d_kw2[:, :])
    kb2_sb = persist.tile([H, 1], F32)
    nc.sync.dma_start(out=kb2_sb[:, :], in_=_col(d_kb2))
    bout_bc = persist.tile([P, DX], F32)
    nc.sync.dma_start(
        out=bout_bc[:, :],
        in_=bass.AP(tensor=d_bout.tensor, offset=d_bout.offset,
                    ap=[[0, P], [1, DX]]))

    tqT_bf = persist.tile([DT, NQ], BF16)
    tkT_bf = persist.tile([DT, NKV], BF16)
    kw1_bf = persist.tile([DT, KHID], BF16)
    nc.vector.tensor_copy(tqT_bf[:, :], tqT[:, :])
    nc.vector.tensor_copy(tkT_bf[:, :], tkT[:, :])
    nc.vector.tensor_copy(kw1_bf[:, :], kw1_sb[:, :])
    kw2_bf = persist.tile([KHID, H], BF16)
    nc.vector.tensor_copy(kw2_bf[:, :], kw2_sb[:, :])

    # kb2 pattern tile: partition (half,h,klo) -> kb2[h]  (DRAM step-0 bcast)
    kb2_pat = persist.tile([P, 1], F32)
    for half in range(2):
        for h in range(H):
            nc.sync.dma_start(
                out=kb2_pat[half * 64 + h * 8: half * 64 + h * 8 + 8, :],
                in_=bass.AP(tensor=d_kb2.tensor, offset=d_kb2.offset + h,
                            ap=[[0, 8], [1, 1]]))

    # E matrix [128=(klo,c), 64=(h,klo)]: E[klo*16+c, h*8+klo] = kw2[c,h]
    E_sb = persist.tile([P, 64], BF16)
    nc.vector.memset(E_sb[:, :], 0.0)
    for klo in range(8):
        nc.sync.dma_start(
            out=_unit(E_sb[klo * 16: klo * 16 + 16, klo::8]),
            in_=_unit(kw2_bf[:, :]))

    ones_sb = persist.tile([P, 1], BF16)
    nc.vector.memset(ones_sb[:, :], 1.0)
    zeros_sb = persist.tile([P, P], BF16)
    nc.vector.memset(zeros_sb[:, :], 0.0)

    # aT[c,q] = kw1^T tqT + kb1 (bf16); b[k,c] = -(tk kw1) (f32, per k-tile)
    aT_ps = psum_a.tile([KHID, NQ], F32)
    for j in range(NQ // 512):
        nc.tensor.matmul(aT_ps[:, j * 512:(j + 1) * 512], kw1_bf[:, :],
                         tqT_bf[:, j * 512:(j + 1) * 512], start=True, stop=True)
    aT_bf = persist.tile([KHID, NQ], BF16)
    nc.scalar.activation(aT_bf[:, :], aT_ps[:, :],
                         mybir.ActivationFunctionType.Identity,
                         bias=kb1_sb[:, :], scale=1.0)
    dram_pool = ctx.enter_context(tc.tile_pool(name="drsc", bufs=1, space="DRAM"))
    aT_dram = dram_pool.tile([KHID, NQ], BF16)
    nc.sync.dma_start(out=aT_dram[:, :], in_=aT_bf[:, :])

    b_sb = persist.tile([P, NKT, KHID], F32)
    for kt in range(NKT):
        b_ps = psum_w.tile([P, KHID], F32, tag="b_ps")
        nc.tensor.matmul(b_ps[:, :], tkT_bf[:, kt * P:(kt + 1) * P],
                         kw1_bf[:, :], start=True, stop=True)
        nc.scalar.activation(b_sb[:, kt, :], b_ps[:, :],
                             mybir.ActivationFunctionType.Copy, scale=-1.0)

    # ================= phase 1: projections =================
    # load weights, cast bf16
    def load_w(dram, name):
        w_f = stage.tile([P, NDXT, INNER], F32, tag="w_f")
        nc.sync.dma_start(out=w_f[:, :, :],
                          in_=dram.rearrange("(t p) i -> p t i", p=P))
        w_b = (persist if name == "wout_bf" else xw_pool).tile(
            [P, NDXT, INNER], BF16, tag=name)
        for t in range(NDXT):
            nc.vector.tensor_copy(w_b[:, t, :], w_f[:, t, :])
        return w_b

    wq_bf = load_w(d_wq, "wq_bf")
    wk_bf = load_w(d_wk, "wk_bf")
    wv_bf = load_w(d_wv, "wv_bf")
    wout_bf = load_w(d_wout, "wout_bf")

    # load x straight, cast bf16, DMA-transpose to xT_bf [128, 4, 1024]
    def load_xT(dram, name):
        xT = xw_pool.tile([P, NDXT, NQ], BF16, tag=name)
        for qt in range(NQ // P):
            x_f = stage.tile([P, DX], F32, tag="x_f")
            nc.sync.dma_start(out=x_f[:, :], in_=dram[qt * P:(qt + 1) * P, :])
            x_b = stage.tile([P, DX], BF16, tag="x_b")
            nc.vector.tensor_copy(x_b[:, :], x_f[:, :])
            for dt_ in range(NDXT):
                nc.sync.dma_start_transpose(
                    out=xT[:, dt_, qt * P:(qt + 1) * P],
                    in_=x_b[:, dt_ * P:(dt_ + 1) * P])
        return xT

    xqT_bf = load_xT(d_xq, "xqT_bf")
    xkT_bf = load_xT(d_xk, "xkT_bf")
    xvT_bf = load_xT(d_xv, "xvT_bf")

    # qT/kT [128, 4, 1024] bf16 (qT folded with SCALE); v [128, 8, 512] bf16
    qT_bf = persist.tile([P, NIT, NQ], BF16)
    kT_bf = persist.tile([P, NIT, NKV], BF16)
    v_bf = [persist.tile([P, INNER], BF16, tag=f"v_bf{kt}", name=f"v_bf{kt}")
            for kt in range(NKT)]
    for it in range(NIT):
        for j in range(NQ // 512):
            pq = psum_w.tile([P, 512], F32, tag="proj_ps")
            pk = psum_w.tile([P, 512], F32, tag="proj_ps")
            for dt_ in range(NDXT):
                nc.tensor.matmul(pq[:, :], wq_bf[:, dt_, it * P:(it + 1) * P],
                                 xqT_bf[:, dt_, j * 512:(j + 1) * 512],
                                 start=(dt_ == 0), stop=(dt_ == NDXT - 1))
            for dt_ in range(NDXT):
                nc.tensor.matmul(pk[:, :], wk_bf[:, dt_, it * P:(it + 1) * P],
                                 xkT_bf[:, dt_, j * 512:(j + 1) * 512],
                                 start=(dt_ == 0), stop=(dt_ == NDXT - 1))
            nc.scalar.activation(qT_bf[:, it, j * 512:(j + 1) * 512], pq[:, :],
                                 mybir.ActivationFunctionType.Copy, scale=SCALE)
            nc.scalar.activation(kT_bf[:, it, j * 512:(j + 1) * 512], pk[:, :],
                                 mybir.ActivationFunctionType.Copy, scale=1.0)
    for kt in range(NKT):
        pv = psum_w.tile([P, INNER], F32, tag="proj_ps")
        for dt_ in range(NDXT):
            nc.tensor.matmul(pv[:, :], xvT_bf[:, dt_, kt * P:(kt + 1) * P],
                             wv_bf[:, dt_, :],
                             start=(dt_ == 0), stop=(dt_ == NDXT - 1))
        nc.vector.tensor_copy(v_bf[kt][:, :], pv[:, :])

    # ================= phase 2: attention per q-chunk =================
    ctx0.close()   # free phase-0/1 transient SBUF + PSUM pools
    ctx2 = contextlib.ExitStack()
    psum_acc = ctx2.enter_context(tc.tile_pool(name="psum_acc", bufs=1, space="PSUM"))
    rshuf_pool = ctx2.enter_context(tc.tile_pool(name="rshuf", bufs=2))
    rk_pool = ctx2.enter_context(tc.tile_pool(name="rk", bufs=2))
    eb_pool = ctx2.enter_context(tc.tile_pool(name="eb", bufs=2))
    ebs_pool = ctx2.enter_context(tc.tile_pool(name="ebs", bufs=2))
    et_pool = ctx2.enter_context(tc.tile_pool(name="et", bufs=4))
    p_pool = ctx2.enter_context(tc.tile_pool(name="pp", bufs=4))
    abc_pool = ctx2.enter_context(tc.tile_pool(name="abc", bufs=1))
    psum_d = ctx2.enter_context(tc.tile_pool(name="psum_d", bufs=1, space="PSUM"))

    outT_sb = persist.tile([P, NIT, NQ], BF16)   # [ (h-pair d), it, q ]

    for qc in range(NQC):
        q0 = qc * QC
        # A_bc [128, c, 512] bf16 broadcast of aT
        A_bc = abc_pool.tile([P, KHID, QC], BF16)
        for c in range(KHID):
            nc.sync.dma_start(
                out=A_bc[:, c, :],
                in_=bass.AP(tensor=aT_dram.tensor,
                            offset=aT_dram[c:c + 1, q0:q0 + QC].offset,
                            ap=[[0, P], [1, QC]]))

        # persistent accumulators for this q-chunk
        av_ps = psum_acc.tile([P, NIT, QC], F32, tag="av")      # 4 banks, 2 heads each
        z_ps = psum_acc.tile([P, 2, QC], F32, tag="z")          # 2 banks, 4 slots each
        for it in range(NIT):
            nc.tensor.matmul(av_ps[:, it, :], zeros_sb[:, :],
                             wout_bf[:, 0, :], start=True, stop=False)
        for zb in range(2):
            nc.tensor.matmul(z_ps[:, zb, :], zeros_sb[:, :],
                             wout_bf[:, 0, :], start=True, stop=False)

        for kt in range(NKT):
            # ---- R production: R_k[k, c, q] = relu(A_bc + b) ----
            R_k = rk_pool.tile([P, KHID, QC], BF16)
            for c in range(KHID):
                nc.vector.tensor_scalar(
                    out=R_k[:, c, :], in0=A_bc[:, c, :],
                    scalar1=b_sb[:, kt, c:c + 1], scalar2=0.0,
                    op0=AX.add, op1=AX.max)
            # ---- R shuffle: -> R_shuf[(klo,c), kgrp, q] ----
            R_shuf = rshuf_pool.tile([P, 16, QC], BF16)
            for kg in range(16):
                nc.gpsimd.dma_start(
                    out=R_shuf[:, kg, :],
                    in_=R_k[kg * 8:(kg + 1) * 8, :, :])
            # ---- E-mm pairs + exp -> E_b_sb [128=(half,h,klo), pair, q] ----
            E_b = eb_pool.tile([P, 8, QC], BF16)
            for pair in range(8):
                bias_ps = psum_d.tile([P, QC], F32, tag="bias_ps")
                nc.tensor.matmul(bias_ps[0:64, :], E_sb[:, :],
                                 R_shuf[:, 2 * pair, :], start=True, stop=True)
                nc.tensor.matmul(bias_ps[64:128, :], E_sb[:, :],
                                 R_shuf[:, 2 * pair + 1, :], start=True, stop=True,
                                 tile_position=(0, 64))
                nc.scalar.activation(E_b[:, pair, :], bias_ps[:, :],
                                     mybir.ActivationFunctionType.Exp,
                                     bias=kb2_pat[:, :], scale=1.0)
            # ---- E_b shuffle -> E_b_shuf [k, h, q] ----
            E_b_shuf = ebs_pool.tile([P, H, QC], BF16)
            for h in range(H):
                for pair in range(8):
                    for half in range(2):
                        eng = nc.sync if (pair % 2 == 0) else nc.scalar
                        eng.dma_start(
                            out=E_b_shuf[pair * 16 + half * 8:
                                         pair * 16 + half * 8 + 8, h, :],
                            in_=E_b[half * 64 + h * 8: half * 64 + h * 8 + 8,
                                    pair, :])
            # ---- per head: QK, exp, P=Et*Eb, AV, Z ----
            for h in range(H):
                it = h // 2
                r0 = (h % 2) * 64
                dots_ps = psum_d.tile([P, QC], F32, tag="dots_ps")
                nc.tensor.matmul(
                    dots_ps[:, :],
                    kT_bf[r0:r0 + 64, it, kt * P:(kt + 1) * P],
                    qT_bf[r0:r0 + 64, it, q0:q0 + QC],
                    start=True, stop=True)
                E_t = et_pool.tile([P, QC], BF16)
                nc.scalar.activation(E_t[:, :], dots_ps[:, :],
                                     mybir.ActivationFunctionType.Exp)
                P_sb = p_pool.tile([P, QC], BF16)
                nc.vector.tensor_mul(P_sb[:, :], E_t[:, :], E_b_shuf[:, h, :])
                nc.tensor.matmul(
                    av_ps[r0:r0 + 64, it, :],
                    v_bf[kt][:, h * 64:(h + 1) * 64], P_sb[:, :],
                    start=False, stop=False,
                    tile_position=(0, r0) if r0 else None)
                zslot = h % 4
                nc.tensor.matmul(
                    z_ps[zslot * 32: zslot * 32 + 1, h // 4, :],
                    ones_sb[:, :], P_sb[:, :],
                    start=False, stop=False,
                    tile_position=(0, zslot * 32) if zslot else None)

        for it in range(NIT):
            nc.tensor.matmul(av_ps[:, it, :], zeros_sb[:, :],
                             wout_bf[:, 0, :], start=False, stop=True)
        for zb in range(2):
            nc.tensor.matmul(z_ps[:, zb, :], zeros_sb[:, :],
                             wout_bf[:, 0, :], start=False, stop=True)
        # ---- normalize: outT = av / Z (recip -> DRAM bounce -> bcast) ----
        zr_dram = dram_pool.tile([H, QC], F32, tag="zr_dram")
        for h in range(H):
            zr_t = work.tile([1, QC], F32, tag="zr_t")
            nc.vector.reciprocal(zr_t[:, :],
                                 z_ps[(h % 4) * 32:(h % 4) * 32 + 1, h // 4, :])
            nc.sync.dma_start(out=zr_dram[h:h + 1, :], in_=zr_t[:, :])
        for h in range(H):
            zr_bc = work.tile([64, QC], F32, tag="zr_bc")
            nc.sync.dma_start(
                out=zr_bc[:, :],
                in_=bass.AP(tensor=zr_dram.tensor,
                            offset=zr_dram[h:h + 1, :].offset,
                            ap=[[0, 64], [1, QC]]))
            r0 = (h % 2) * 64
            nc.vector.tensor_mul(outT_sb[r0:r0 + 64, h // 2, q0:q0 + QC],
                                 av_ps[r0:r0 + 64, h // 2, :], zr_bc[:, :])

    # ================= phase 3: output projection =================
    ctx2.close()   # free phase-2 pools
    psum_o = ctx.enter_context(tc.tile_pool(name="psum_o", bufs=2, space="PSUM"))
    for qt in range(NQ // P):
        op = psum_o.tile([P, DX], F32, tag="op")
        for it in range(NIT):
            nc.tensor.matmul(op[:, :],
                             outT_sb[:, it, qt * P:(qt + 1) * P],
                             wout_bf[:, it, :],
                             start=(it == 0), stop=(it == NIT - 1))
        o_sb = work.tile([P, DX], F32, tag="o_sb")
        nc.vector.tensor_add(o_sb[:, :], op[:, :], bout_bc[:, :])
        nc.sync.dma_start(out=d_out[qt * P:(qt + 1) * P, :], in_=o_sb[:, :])

    ctx.close()


_NC_CACHE = None


def _get_nc():
    global _NC_CACHE
    if _NC_CACHE is None:
        nc = bacc.Bacc("TRN2", target_bir_lowering=False, debug=False,
                       enable_asserts=False, num_devices=M)
        build_kernel(nc)
        nc.compile()
        _NC_CACHE = nc
    return _NC_CACHE


def kernel(**inputs):
    nc = _get_nc()
    shared = {n: np.ascontiguousarray(np.asarray(inputs[n], dtype=np.float32))
              for n in ["w_q", "w_k", "w_v", "w_out", "b_out",
                        "kw1", "kb1", "kw2", "kb2"]}
    in_maps = []
    for i in range(M):
        m = dict(shared)
        for n in ["xq", "xk", "xv", "tq", "tk"]:
            m[n] = np.ascontiguousarray(np.asarray(inputs[n][i], dtype=np.float32))
        in_maps.append(m)
    res = bass_utils.run_bass_kernel_spmd(nc, in_maps, core_ids=list(range(M)))
    out = np.stack([res.results[i]["out"] for i in range(M)], axis=0)
    return out.astype(np.float32)


if __name__ == "__main__":
    import reference
    inputs = {k: np.asarray(v) for k, v in reference.setup_inputs().items()}
    out = kernel(**inputs)
    print("out", out.shape, out.dtype)
